# revision 48
# baseline (speedup 1.0000x reference)
"""Distributed Bass kernel for nn_Attention (dense transformer attention block).

Tensor-parallel over heads across 8 TRN2 NeuronCores:
  - each core owns 4 heads: its slice of W_pack (QKV) and the matching
    512 input channels of o_proj,
  - attention (RoPE + causal softmax) is computed fully locally per head,
  - attention outputs (UNNORMALIZED, with per-row softmax denominators kept
    separately) are AllGathered in fp16 in two q-halves per batch; each core
    computes a distinct 512-column slice of the o_proj output with the
    1/denominator folded into the psum evacuation as a per-partition scale.

Compute dtype: fp16 matmuls with f32 PSUM accumulation, f32 softmax logits.
Scores are computed transposed ([k, q] layout) so the softmax exp writes
P^T directly and PV needs no transpose. Softmax denominators come from
DVE pair-accumulation of the exp tiles into an fp16 accumulator plus two
small ones-matmuls per group (instead of one ones-matmul per k-tile),
keeping the tensor engine (the global bottleneck, power-throttled to
~81% duty) on pure model FLOPs.

Schedule: stage1 (QKV+RoPE, PE-saturated) -> attention b0 (qc 3,2 ->
AllGather half1, qc 1,0 -> AllGather half0) -> attention b1 (kqv tiles
prefetched) -> o_proj b0 -> o_proj b1, with the AllGathers flying under
attention/o_proj compute.
"""

import sys
import types
import math

sys.path.insert(0, "/opt/trn_rl_repo")

import numpy as np

from concourse import bacc, tile, mybir
from concourse.bass_utils import run_bass_kernel_spmd

FP16 = mybir.dt.float16
F32 = mybir.dt.float32

B = 2
S = 2048
H = 4096
NH = 32
D = 128
T = B * S
N_CORES = 8
HEADS_PER_CORE = NH // N_CORES          # 4
CH = HEADS_PER_CORE * D                 # 512 channels per core
BASE = 10000.0
NEG = np.float32(np.finfo(np.float32).min)

# mask-block ops (per [q-chunk=512, k-tile=128] block, scoresT layout)
SKIP, NOMASK, DIAG, DMAMASK = 0, 1, 2, 3

QC = S // 512                            # 4 q-chunks per batch
KT = S // 128                            # 16 k-tiles per batch

_cache = {}
last_run_info = {}


def _ensure_trace_hook():
    """Register the NTFF profile hook missing from this image's antenv."""
    if "antenv.axon_hooks" in sys.modules:
        return
    try:
        from trn_agent_boot.trn_boot import _ntff_profile_via_ctypes

        hook = _ntff_profile_via_ctypes("/opt/axon/libaxon_pjrt.so")
        mod = types.ModuleType("antenv.axon_hooks")
        mod.get_axon_ntff_profile_hook = lambda: hook
        mod.set_axon_ntff_profile_hook = lambda h: None
        sys.modules["antenv.axon_hooks"] = mod
        from concourse import bass_utils

        bass_utils.upload_artifacts = lambda tmpdir: tmpdir
    except Exception:
        pass


def _classify_mask(mask):
    """Per (b, q-chunk 512, k-tile 128) block op for the scoresT layout."""
    ops = np.empty((B, QC, KT), dtype=np.int32)
    karr = np.arange(128)
    qarr = np.arange(512)
    need_dma = False
    for b in range(B):
        mb = np.asarray(mask[b, 0])
        for qc in range(QC):
            qs = qc * 512
            for kt in range(KT):
                ks = kt * 128
                sub = mb[qs : qs + 512, ks : ks + 128]  # [q, k]
                if np.all(sub <= -1e30):
                    ops[b, qc, kt] = SKIP
                elif not sub.any():
                    ops[b, qc, kt] = NOMASK
                else:
                    delta = ks - qs
                    if 0 <= delta <= 384:
                        pat = np.where(
                            (delta + karr[None, :]) > qarr[:, None], NEG, np.float32(0)
                        )
                        if np.array_equal(sub, pat):
                            ops[b, qc, kt] = DIAG
                            continue
                    ops[b, qc, kt] = DMAMASK
                    need_dma = True
    return ops, need_dma


def _build(ops, need_dma):
    nc = bacc.Bacc(None, target_bir_lowering=False)

    TSL = S // N_CORES                       # 256: this core's token slice per batch

    x_t = nc.declare_dram_parameter("x_t", [H, T], FP16, isOutput=False)
    wqk = nc.declare_dram_parameter("wqk", [H, 2 * CH], FP16, isOutput=False)
    wv = nc.declare_dram_parameter("wv", [H, CH], FP16, isOutput=False)
    wo = nc.declare_dram_parameter("wo", [H, H], FP16, isOutput=False)
    tabs = nc.declare_dram_parameter("tabs", [4, D, T], FP16, isOutput=False)
    maskT = None
    if need_dma:
        maskT = nc.declare_dram_parameter("maskT", [B, S, S], F32, isOutput=False)
    out = nc.declare_dram_parameter("out", [B * TSL, H], F32, isOutput=True)

    ones_mat_np = np.ones((D, D), dtype=np.float16)
    # maskbin[k, j] = 0 if k > j - 384 else 1  (slice [384-delta : 896-delta])
    j = np.arange(896)
    maskbin_np = np.where(np.arange(D)[:, None] > (j[None, :] - 384), 0.0, 1.0).astype(
        np.float16
    )

    rg = [list(range(N_CORES))]
    NHT = H // D  # 32 h-tiles

    with tile.TileContext(nc) as tc:
        with (
            tc.tile_pool(name="dram", bufs=1, space="DRAM") as dram,
            tc.tile_pool(name="const", bufs=1) as constp,
            tc.tile_pool(name="wpool", bufs=1) as wpool,
        ):
            wpool_bv = tc.alloc_tile_pool(name="wpool_bv", bufs=1)
            kqv_pre = tc.alloc_tile_pool(name="kqv_pre", bufs=2, side="right")
            qt_d = dram.tile([CH, T], FP16, tag="qt_d")
            kt_d = dram.tile([CH, T], FP16, tag="kt_d")
            v_d = dram.tile([T, CH], FP16, tag="v_d")
            # AllToAll staging: chunk j of a2a_in = this core's attention
            # for token-slice j; after the exchange, a2a_out viewed as
            # [4096, 256] is the FULL-hidden attention for THIS core's slice.
            a2a_in = [
                dram.tile([N_CORES, CH, TSL], FP16, tag=f"a2a_in{b}",
                          name=f"a2a_in{b}")
                for b in range(B)
            ]
            a2a_out = [
                dram.tile([N_CORES, CH, TSL], FP16, tag=f"a2a_out{b}",
                          name=f"a2a_out{b}")
                for b in range(B)
            ]

            # ------------- stage 1: QKV projection + RoPE (single X pass) ----
            wqk_a = wpool.tile([D, NHT // 2, 2 * CH], FP16, tag="wqkA", name="wqk_a")
            wqk_b = wpool_bv.tile([D, NHT // 2, 2 * CH], FP16, tag="wqkB", name="wqk_b")
            wv_sb = wpool_bv.tile([D, NHT, CH], FP16, tag="wv")

            def wqk_at(h):
                return wqk_a[:, h, :] if h < NHT // 2 else wqk_b[:, h - NHT // 2, :]

            with (
                tc.tile_pool(name="xpool", bufs=2) as xpool,
                tc.tile_pool(name="tpool", bufs=1) as tpool,
                tc.tile_pool(name="rpool", bufs=2) as rpool,
                tc.tile_pool(name="qkout", bufs=2) as qkout,
                tc.tile_pool(name="ps1", bufs=6, space="PSUM") as ps1,
            ):
                # chunk-0 x tiles allocated up front so their DMAs can be
                # interleaved with the weight quarters (fast PE start)
                x_a0 = xpool.tile([D, NHT // 2, 512], FP16, tag="xA", name="x_a0")
                x_b0 = xpool.tile([D, NHT // 2, 512], FP16, tag="xB", name="x_b0")
                for q in range(4):
                    r0 = q * 512
                    nc.sync.dma_start(
                        wqk_a[:, 4 * q : 4 * q + 4, :],
                        wqk[r0 : r0 + 512].rearrange("(ho p) c -> p ho c", p=D),
                    )
                    nc.sync.dma_start(
                        x_a0[:, 4 * q : 4 * q + 4, :],
                        x_t[r0 : r0 + 512, 0:512].rearrange("(ho p) t -> p ho t", p=D),
                    )
                for q in range(4):
                    r0 = H // 2 + q * 512
                    nc.sync.dma_start(
                        wqk_b[:, 4 * q : 4 * q + 4, :],
                        wqk[r0 : r0 + 512].rearrange("(ho p) c -> p ho c", p=D),
                    )
                    nc.sync.dma_start(
                        x_b0[:, 4 * q : 4 * q + 4, :],
                        x_t[r0 : r0 + 512, 0:512].rearrange("(ho p) t -> p ho t", p=D),
                    )
                tb0 = tpool.tile([D, 4, 512], FP16, tag="tb", name="tb0")
                for ti in range(4):
                    nc.sync.dma_start(tb0[:, ti, :], tabs[ti, :, 0:512])
                nc.sync.dma_start(
                    wv_sb[:, 0 : NHT // 2, :],
                    wv[: H // 2].rearrange("(ho p) c -> p ho c", p=D),
                )
                nc.sync.dma_start(
                    wv_sb[:, NHT // 2 :, :],
                    wv[H // 2 :].rearrange("(ho p) c -> p ho c", p=D),
                )
                ones_mat = constp.tile([D, D], FP16, tag="ones_mat")
                nc.sync.dma_start(ones_mat[:], nc.inline_tensor(ones_mat_np, "ones_mat_c")[:])
                maskbin = constp.tile([D, 896], FP16, tag="maskbin")
                nc.sync.dma_start(maskbin[:], nc.inline_tensor(maskbin_np, "maskbin_c")[:])

                for tci in range(T // 512):
                    t0 = tci * 512
                    if tci == 0:
                        x_a, x_b, tb = x_a0, x_b0, tb0
                    else:
                        x_a = xpool.tile([D, NHT // 2, 512], FP16, tag="xA",
                                         name=f"x_a{tci}")
                        nc.sync.dma_start(
                            x_a[:],
                            x_t[: H // 2, t0 : t0 + 512].rearrange(
                                "(ho p) t -> p ho t", p=D
                            ),
                        )
                        x_b = xpool.tile([D, NHT // 2, 512], FP16, tag="xB",
                                         name=f"x_b{tci}")
                        nc.sync.dma_start(
                            x_b[:],
                            x_t[H // 2 :, t0 : t0 + 512].rearrange(
                                "(ho p) t -> p ho t", p=D
                            ),
                        )
                        tb = tpool.tile([D, 4, 512], FP16, tag="tb", name=f"tb{tci}")
                        for ti in range(4):
                            nc.sync.dma_start(tb[:, ti, :], tabs[ti, :, t0 : t0 + 512])

                    def x_at(h, x_a=x_a, x_b=x_b):
                        return x_a[:, h, :] if h < NHT // 2 else x_b[:, h - NHT // 2, :]

                    for ct in range(2 * CH // D):  # 0-3: q heads, 4-7: k heads
                        psum = ps1.tile([D, 512], F32, tag="ps1")
                        for h in range(NHT):
                            nc.tensor.matmul(
                                psum[:],
                                wqk_at(h)[:, ct * D : (ct + 1) * D],
                                x_at(h),
                                start=(h == 0),
                                stop=(h == NHT - 1),
                            )
                        is_q = ct < HEADS_PER_CORE
                        cos_i = 0 if is_q else 2
                        sin_i = 1 if is_q else 3
                        tmp1 = rpool.tile([D, 512], F32, tag="tmp1")
                        nc.vector.tensor_mul(tmp1[:], psum[:], tb[:, cos_i, :])
                        tmp2 = rpool.tile([D, 512], F32, tag="tmp2")
                        nc.vector.tensor_mul(
                            tmp2[0:64, :], psum[64:128, :], tb[0:64, sin_i, :]
                        )
                        nc.vector.tensor_mul(
                            tmp2[64:128, :], psum[0:64, :], tb[64:128, sin_i, :]
                        )
                        qk_16 = qkout.tile([D, 512], FP16, tag="qk_16")
                        nc.vector.tensor_add(qk_16[:], tmp1[:], tmp2[:])
                        head = ct % HEADS_PER_CORE
                        dst = qt_d if is_q else kt_d
                        nc.sync.dma_start(
                            dst[head * D : (head + 1) * D, t0 : t0 + 512], qk_16[:]
                        )

                    for ts in range(4):  # V: [t, ch] layout
                        psum = ps1.tile([D, 512], F32, tag="ps1", name="psum_v")
                        for h in range(NHT):
                            nc.tensor.matmul(
                                psum[:],
                                x_at(h)[:, ts * D : (ts + 1) * D],
                                wv_sb[:, h, :],
                                start=(h == 0),
                                stop=(h == NHT - 1),
                            )
                        v_16 = qkout.tile([D, CH], FP16, tag="v_16")
                        nc.scalar.activation(
                            v_16[:], psum[:], mybir.ActivationFunctionType.Copy
                        )
                        nc.sync.dma_start(
                            v_d[t0 + ts * D : t0 + (ts + 1) * D, :], v_16[:]
                        )

            wpool_bv.release()
            kqv = tc.alloc_tile_pool(name="kqv", bufs=6)

            def load_head(b, head, pool):
                k_sb = pool.tile([D, S], FP16, tag="k_sb", name=f"k_sb{b}_{head}")
                nc.sync.dma_start(
                    k_sb[:], kt_d[head * D : (head + 1) * D, b * S : (b + 1) * S]
                )
                q_sb = pool.tile([D, S], FP16, tag="q_sb", name=f"q_sb{b}_{head}")
                nc.sync.dma_start(
                    q_sb[:], qt_d[head * D : (head + 1) * D, b * S : (b + 1) * S]
                )
                v_sb = pool.tile([D, KT, D], FP16, tag="v_sb", name=f"v_sb{b}_{head}")
                nc.sync.dma_start(
                    v_sb[:],
                    v_d[b * S : (b + 1) * S, head * D : (head + 1) * D].rearrange(
                        "(o p) c -> p o c", p=D
                    ),
                )
                return k_sb, q_sb, v_sb

            # b0 heads 0/1 live in the pre-pool (virgin SBUF, loads flow
            # during the stage-1 tensor tail); everything else in kqv.
            # All loads are emitted up front so none of them queue behind
            # the AllGather rings in the DMA queues.
            tiles = {}
            tiles[(0, 0)] = load_head(0, 0, kqv_pre)
            tiles[(0, 1)] = load_head(0, 1, kqv_pre)
            for bb, hh in [(0, 2), (0, 3), (1, 0), (1, 1), (1, 2), (1, 3)]:
                tiles[(bb, hh)] = load_head(bb, hh, kqv)

            # ------------- stage 2: attention + chunked AllGather -------------
            with (
                tc.tile_pool(name="ppool", bufs=10) as ppool,
                tc.tile_pool(name="accp", bufs=4) as accp,
                tc.tile_pool(name="mpool", bufs=2) as mpool,
                tc.tile_pool(name="epi", bufs=2) as epi,
                tc.tile_pool(name="ps_s", bufs=2, space="PSUM") as ps_s,
                tc.tile_pool(name="ps_av", bufs=3, space="PSUM") as ps_av,
                tc.tile_pool(name="ps_sbc", bufs=1, space="PSUM") as ps_sbc,
            ):
                qe, qm = [], []

                def drain(q, keep=0):
                    while len(q) > keep:
                        q.pop(0)()

                def group_gen(b, head, qc, tiles):
                    k_sb, q_sb, v_sb = tiles
                    diag = [kt for kt in range(KT) if ops[b, qc, kt] == DIAG]
                    dmam = [kt for kt in range(KT) if ops[b, qc, kt] == DMAMASK]
                    plain = [kt for kt in range(KT) if ops[b, qc, kt] == NOMASK]
                    kts = diag + dmam + plain
                    n_kt = len(kts)
                    psum_av = ps_av.tile([D, 512], F32, tag="av", name=f"av{b}{head}{qc}")
                    acc2 = accp.tile([D, 2, 512], FP16, tag="acc2",
                                     name=f"acc2{b}{head}{qc}")
                    gi = 0
                    for p0 in range(0, n_kt, 2):
                        pair = kts[p0 : p0 + 2]
                        np_ = len(pair)
                        psum_s = ps_s.tile(
                            [D, 2, 512], F32, tag="s", name=f"s{b}{head}{qc}{p0}"
                        )
                        diag_slices = []
                        for sl, kt in enumerate(pair):
                            nc.tensor.matmul(
                                psum_s[:, sl, :],
                                k_sb[:, kt * D : (kt + 1) * D],
                                q_sb[:, qc * 512 : (qc + 1) * 512],
                                start=True,
                                stop=True,
                            )
                            op = ops[b, qc, kt]
                            if op == DIAG:
                                diag_slices.append((sl, kt * D - qc * 512))
                            elif op == DMAMASK:
                                mt = mpool.tile([D, 512], F32, tag="mt", name="mt")
                                nc.sync.dma_start(
                                    mt[:],
                                    maskT[
                                        b,
                                        kt * D : (kt + 1) * D,
                                        qc * 512 : (qc + 1) * 512,
                                    ],
                                )
                                nc.vector.tensor_add(
                                    psum_s[:, sl, :], psum_s[:, sl, :], mt[:]
                                )

                        cell = []
                        first = p0 == 0

                        def exp_step(psum_s=psum_s, np_=np_, cell=cell,
                                     diag_slices=diag_slices, first=first,
                                     acc2=acc2):
                            pexp = ppool.tile([D, 2, 512], FP16, tag="pexp", name="pexp")
                            nc.scalar.activation(
                                pexp[:, 0:np_, :],
                                psum_s[:, 0:np_, :],
                                mybir.ActivationFunctionType.Exp,
                            )
                            for sl, delta in diag_slices:
                                nc.gpsimd.tensor_mul(
                                    pexp[:, sl, :],
                                    pexp[:, sl, :],
                                    maskbin[:, 384 - delta : 896 - delta],
                                )
                            eng = nc.vector
                            if first:
                                eng.tensor_copy(
                                    acc2[:, 0:np_, :], pexp[:, 0:np_, :]
                                )
                            else:
                                eng.tensor_add(
                                    acc2[:, 0:np_, :], acc2[:, 0:np_, :],
                                    pexp[:, 0:np_, :],
                                )
                            cell.append(pexp)

                        def mm_step(
                            psum_av=psum_av,
                            gi=gi,
                            pair=pair,
                            n_kt=n_kt,
                            v_sb=v_sb,
                            cell=cell,
                        ):
                            pexp = cell[0]
                            for sl, kt in enumerate(pair):
                                i = gi + sl
                                nc.tensor.matmul(
                                    psum_av[:],
                                    v_sb[:, kt, :],
                                    pexp[:, sl, :],
                                    start=(i == 0),
                                    stop=(i == n_kt - 1),
                                )

                        gi += np_
                        yield (exp_step, mm_step)

                    def epilogue(psum_av=psum_av, acc2=acc2, n_kt=n_kt):
                        n_sl = 2 if n_kt >= 2 else 1
                        psum_sbc = ps_sbc.tile(
                            [D, 512], F32, tag="sbc", name=f"sbc{b}{head}{qc}"
                        )
                        for sl in range(n_sl):
                            nc.tensor.matmul(
                                psum_sbc[:],
                                ones_mat[:],
                                acc2[:, sl, :],
                                start=(sl == 0),
                                stop=(sl == n_sl - 1),
                            )
                        bc_sb = epi.tile([D, 512], F32, tag="bc_sb", name="bc_sb")
                        nc.vector.reciprocal_approx_fast(bc_sb[:], psum_sbc[:])
                        attn_sb = epi.tile([D, 512], FP16, tag="attn_sb", name="attn_sb")
                        nc.vector.tensor_mul(attn_sb[:], psum_av[:], bc_sb[:])
                        for j in range(2):
                            nc.sync.dma_start(
                                a2a_in[b][
                                    2 * qc + j,
                                    head * D : (head + 1) * D,
                                    :,
                                ],
                                attn_sb[:, j * TSL : (j + 1) * TSL],
                            )

                    yield (None, epilogue)

                def run_section(gens):
                    active = []
                    queue = list(gens)
                    while queue or active:
                        while len(active) < 2 and queue:
                            active.append(queue.pop(0))
                        progressed = []
                        for g in active:
                            item = next(g, None)
                            if item is None:
                                continue
                            e, m = item
                            if e is not None:
                                qe.append(e)
                            qm.append(m)
                            drain(qe, 2)
                            drain(qm, 10)
                            progressed.append(g)
                        active = [g for g in active if g in progressed]
                    drain(qe)
                    drain(qm)

                for b in range(B):
                    run_section(
                        [
                            group_gen(b, head, qc, tiles[(b, head)])
                            for qc in (3, 2, 1, 0)
                            for head in range(HEADS_PER_CORE)
                        ]
                    )
                    nc.gpsimd.collective_compute(
                        "AllToAll",
                        mybir.AluOpType.bypass,
                        replica_groups=rg,
                        ins=[a2a_in[b].opt()],
                        outs=[a2a_out[b].opt()],
                    )

            kqv.release()
            kqv_pre.release()

            # ------------- stage 3: token-sharded o_proj, streamed W_o --------
            # Each core computes its own 256-token slice (per batch) of the
            # FULL-width o_proj from the AllToAll'd activations; W_o streams
            # through SBUF in 512-column chunks with no collective deps.
            with (
                tc.tile_pool(name="apool", bufs=1) as apool,
                tc.tile_pool(name="wop", bufs=3) as wop,
                tc.tile_pool(name="oppool", bufs=3) as oppool,
                tc.tile_pool(name="ps3", bufs=3, space="PSUM") as ps3,
            ):
                def load_att_f(b):
                    af = apool.tile([D, NHT, TSL], FP16, tag=f"att_f{b}",
                                    name=f"att_f{b}")
                    nc.sync.dma_start(
                        af[:],
                        a2a_out[b][:, :, :].rearrange(
                            "k (co p) t -> p (k co) t", p=D
                        ),
                    )
                    return af

                att_f = [load_att_f(0), None]
                NOC = H // 512                   # 8 o-column chunks
                for oc in range(NOC):
                    wo_c = wop.tile([D, NHT, 512], FP16, tag="wo_c")
                    nc.sync.dma_start(
                        wo_c[:],
                        wo[:, oc * 512 : (oc + 1) * 512].rearrange(
                            "(co p) o -> p co o", p=D
                        ),
                    )
                    if oc == 0:
                        # emitted after the first W_o chunk so its wait on the
                        # b1 AllToAll can't head-of-line-block the weight stream
                        att_f[1] = load_att_f(1)
                    for b in range(B):
                        for tl in range(TSL // D):
                            psum_o = ps3.tile([D, 512], F32, tag="ps_o")
                            for ct in range(NHT):
                                nc.tensor.matmul(
                                    psum_o[:],
                                    att_f[b][:, ct, tl * D : (tl + 1) * D],
                                    wo_c[:, ct, :],
                                    start=(ct == 0),
                                    stop=(ct == NHT - 1),
                                )
                            o_sb = oppool.tile([D, 512], F32, tag="o_sb")
                            nc.scalar.activation(
                                o_sb[:],
                                psum_o[:],
                                mybir.ActivationFunctionType.Copy,
                            )
                            nc.sync.dma_start(
                                out[
                                    b * TSL + tl * D : b * TSL + (tl + 1) * D,
                                    oc * 512 : (oc + 1) * 512,
                                ],
                                o_sb[:],
                            )

    nc.compile()
    return nc, maskT is not None


def kernel(hidden_states, attention_mask, position_ids, W_pack, W_o):
    _ensure_trace_hook()
    hidden_states = np.asarray(hidden_states, dtype=np.float32)
    attention_mask = np.asarray(attention_mask, dtype=np.float32)
    position_ids = np.asarray(position_ids)
    W_pack = np.asarray(W_pack, dtype=np.float32)
    W_o = np.asarray(W_o, dtype=np.float32)

    ops, need_dma = _classify_mask(attention_mask)

    key = (ops.tobytes(), need_dma)
    if key not in _cache:
        _cache.clear()
        _cache[key] = _build(ops, need_dma)
    nc, has_mask_param = _cache[key]

    # ---- host-side prep ----
    X_T = np.ascontiguousarray(hidden_states.reshape(T, H).T).astype(np.float16)

    # RoPE tables (position-gathered), transposed to [d, t]; scale folded into Q's.
    pos = position_ids.reshape(T).astype(np.float32)
    inv_freq = (1.0 / (BASE ** (np.arange(0, D, 2, dtype=np.float32) / D))).astype(
        np.float32
    )
    ang = pos[:, None] * inv_freq[None, :]          # [T, 64]
    ang = np.concatenate([ang, ang], axis=1)         # [T, 128]
    cos = np.cos(ang).astype(np.float32)
    sin = np.sin(ang).astype(np.float32)
    sin_signed = sin.copy()
    sin_signed[:, :64] *= -1.0                       # rows d<64 multiply -q[d+64]
    isd = np.float32(1.0 / math.sqrt(D))
    tabs = np.stack(
        [
            (cos * isd).T,
            (sin_signed * isd).T,
            cos.T,
            sin_signed.T,
        ]
    ).astype(np.float16)                             # [4, 128, T]
    tabs = np.ascontiguousarray(tabs)

    maskT_np = None
    if has_mask_param:
        maskT_np = np.ascontiguousarray(
            np.transpose(attention_mask[:, 0], (0, 2, 1))
        ).astype(np.float32)                         # [B, S(k), S(q)]

    wo_full = np.ascontiguousarray(W_o.T).astype(np.float16)  # [H(in ch), H(out)]
    in_maps = []
    for c in range(N_CORES):
        qr = slice(c * CH, (c + 1) * CH)
        kr = slice(H + c * CH, H + (c + 1) * CH)
        vr = slice(2 * H + c * CH, 2 * H + (c + 1) * CH)
        wqk_c = np.ascontiguousarray(
            np.concatenate([W_pack[qr], W_pack[kr]], axis=0).T
        ).astype(np.float16)                         # [H, 1024]
        wv_c = np.ascontiguousarray(W_pack[vr].T).astype(np.float16)  # [H, 512]
        m = {"x_t": X_T, "wqk": wqk_c, "wv": wv_c, "wo": wo_full, "tabs": tabs}
        if has_mask_param:
            m["maskT"] = maskT_np
        in_maps.append(m)

    import os

    trace = bool(os.environ.get("BASS_TRACE"))
    res = run_bass_kernel_spmd(
        nc, in_maps, core_ids=list(range(N_CORES)), trace=trace
    )
    last_run_info["exec_time_ns"] = res.exec_time_ns
    last_run_info["profile_json"] = getattr(res, "profile_json", None)

    TSL = S // N_CORES
    full = np.empty((B, S, H), dtype=np.float32)
    for c in range(N_CORES):
        o = res.results[c]["out"].reshape(B, TSL, H)
        full[:, c * TSL : (c + 1) * TSL, :] = o
    return full


# revision 49
# speedup vs baseline: 1.0177x; 1.0177x over previous
"""Distributed Bass kernel for nn_Attention (dense transformer attention block).

Tensor-parallel over heads across 8 TRN2 NeuronCores:
  - each core owns 4 heads: its slice of W_pack (QKV) and the matching
    512 input channels of o_proj,
  - attention (RoPE + causal softmax) is computed fully locally per head,
  - attention outputs (UNNORMALIZED, with per-row softmax denominators kept
    separately) are AllGathered in fp16 in two q-halves per batch; each core
    computes a distinct 512-column slice of the o_proj output with the
    1/denominator folded into the psum evacuation as a per-partition scale.

Compute dtype: fp16 matmuls with f32 PSUM accumulation, f32 softmax logits.
Scores are computed transposed ([k, q] layout) so the softmax exp writes
P^T directly and PV needs no transpose. Softmax denominators come from
DVE pair-accumulation of the exp tiles into an fp16 accumulator plus two
small ones-matmuls per group (instead of one ones-matmul per k-tile),
keeping the tensor engine (the global bottleneck, power-throttled to
~81% duty) on pure model FLOPs.

Schedule: stage1 (QKV+RoPE, PE-saturated) -> attention b0 (qc 3,2 ->
AllGather half1, qc 1,0 -> AllGather half0) -> attention b1 (kqv tiles
prefetched) -> o_proj b0 -> o_proj b1, with the AllGathers flying under
attention/o_proj compute.
"""

import sys
import types
import math

sys.path.insert(0, "/opt/trn_rl_repo")

import numpy as np

from concourse import bacc, tile, mybir
from concourse.bass_utils import run_bass_kernel_spmd

FP16 = mybir.dt.float16
F32 = mybir.dt.float32

B = 2
S = 2048
H = 4096
NH = 32
D = 128
T = B * S
N_CORES = 8
HEADS_PER_CORE = NH // N_CORES          # 4
CH = HEADS_PER_CORE * D                 # 512 channels per core
BASE = 10000.0
NEG = np.float32(np.finfo(np.float32).min)

# mask-block ops (per [q-chunk=512, k-tile=128] block, scoresT layout)
SKIP, NOMASK, DIAG, DMAMASK = 0, 1, 2, 3

QC = S // 512                            # 4 q-chunks per batch
KT = S // 128                            # 16 k-tiles per batch

_cache = {}
last_run_info = {}


def _ensure_trace_hook():
    """Register the NTFF profile hook missing from this image's antenv."""
    if "antenv.axon_hooks" in sys.modules:
        return
    try:
        from trn_agent_boot.trn_boot import _ntff_profile_via_ctypes

        hook = _ntff_profile_via_ctypes("/opt/axon/libaxon_pjrt.so")
        mod = types.ModuleType("antenv.axon_hooks")
        mod.get_axon_ntff_profile_hook = lambda: hook
        mod.set_axon_ntff_profile_hook = lambda h: None
        sys.modules["antenv.axon_hooks"] = mod
        from concourse import bass_utils

        bass_utils.upload_artifacts = lambda tmpdir: tmpdir
    except Exception:
        pass


def _classify_mask(mask):
    """Per (b, q-chunk 512, k-tile 128) block op for the scoresT layout."""
    ops = np.empty((B, QC, KT), dtype=np.int32)
    karr = np.arange(128)
    qarr = np.arange(512)
    need_dma = False
    for b in range(B):
        mb = np.asarray(mask[b, 0])
        for qc in range(QC):
            qs = qc * 512
            for kt in range(KT):
                ks = kt * 128
                sub = mb[qs : qs + 512, ks : ks + 128]  # [q, k]
                if np.all(sub <= -1e30):
                    ops[b, qc, kt] = SKIP
                elif not sub.any():
                    ops[b, qc, kt] = NOMASK
                else:
                    delta = ks - qs
                    if 0 <= delta <= 384:
                        pat = np.where(
                            (delta + karr[None, :]) > qarr[:, None], NEG, np.float32(0)
                        )
                        if np.array_equal(sub, pat):
                            ops[b, qc, kt] = DIAG
                            continue
                    ops[b, qc, kt] = DMAMASK
                    need_dma = True
    return ops, need_dma


def _build(ops, need_dma):
    nc = bacc.Bacc(None, target_bir_lowering=False)

    TSL = S // N_CORES                       # 256: this core's token slice per batch

    x_t = nc.declare_dram_parameter("x_t", [H, T], FP16, isOutput=False)
    wqk = nc.declare_dram_parameter("wqk", [H, 2 * CH], FP16, isOutput=False)
    wv = nc.declare_dram_parameter("wv", [H, CH], FP16, isOutput=False)
    wo = nc.declare_dram_parameter("wo", [H, H], FP16, isOutput=False)
    tabs = nc.declare_dram_parameter("tabs", [4, D, T], FP16, isOutput=False)
    maskT = None
    if need_dma:
        maskT = nc.declare_dram_parameter("maskT", [B, S, S], F32, isOutput=False)
    out = nc.declare_dram_parameter("out", [B * TSL, H], F32, isOutput=True)

    ones_mat_np = np.ones((D, D), dtype=np.float16)
    # maskbin[k, j] = 0 if k > j - 384 else 1  (slice [384-delta : 896-delta])
    j = np.arange(896)
    maskbin_np = np.where(np.arange(D)[:, None] > (j[None, :] - 384), 0.0, 1.0).astype(
        np.float16
    )

    rg = [list(range(N_CORES))]
    NHT = H // D  # 32 h-tiles

    with tile.TileContext(nc) as tc:
        with (
            tc.tile_pool(name="dram", bufs=1, space="DRAM") as dram,
            tc.tile_pool(name="const", bufs=1) as constp,
            tc.tile_pool(name="wpool", bufs=1) as wpool,
        ):
            wpool_bv = tc.alloc_tile_pool(name="wpool_bv", bufs=1)
            kqv_pre = tc.alloc_tile_pool(name="kqv_pre", bufs=2, side="right")
            qt_d = dram.tile([CH, T], FP16, tag="qt_d")
            kt_d = dram.tile([CH, T], FP16, tag="kt_d")
            v_d = dram.tile([T, CH], FP16, tag="v_d")
            # AllToAll staging: chunk j of a2a_in = this core's attention
            # for token-slice j; after the exchange, a2a_out viewed as
            # [4096, 256] is the FULL-hidden attention for THIS core's slice.
            a2a_in = [
                dram.tile([N_CORES, CH, TSL], FP16, tag=f"a2a_in{b}",
                          name=f"a2a_in{b}")
                for b in range(B)
            ]
            a2a_out = [
                dram.tile([N_CORES, CH, TSL], FP16, tag=f"a2a_out{b}",
                          name=f"a2a_out{b}")
                for b in range(B)
            ]

            # ------------- stage 1: QKV projection + RoPE (single X pass) ----
            wqk_a = wpool.tile([D, NHT // 2, 2 * CH], FP16, tag="wqkA", name="wqk_a")
            wqk_b = wpool_bv.tile([D, NHT // 2, 2 * CH], FP16, tag="wqkB", name="wqk_b")
            wv_sb = wpool_bv.tile([D, NHT, CH], FP16, tag="wv")

            def wqk_at(h):
                return wqk_a[:, h, :] if h < NHT // 2 else wqk_b[:, h - NHT // 2, :]

            with (
                tc.tile_pool(name="xpool", bufs=2) as xpool,
                tc.tile_pool(name="tpool", bufs=1) as tpool,
                tc.tile_pool(name="rpool", bufs=2) as rpool,
                tc.tile_pool(name="qkout", bufs=2) as qkout,
                tc.tile_pool(name="ps1", bufs=6, space="PSUM") as ps1,
            ):
                # chunk-0 x tiles allocated up front so their DMAs can be
                # interleaved with the weight quarters (fast PE start)
                x_a0 = xpool.tile([D, NHT // 2, 512], FP16, tag="xA", name="x_a0")
                x_b0 = xpool.tile([D, NHT // 2, 512], FP16, tag="xB", name="x_b0")
                for q in range(4):
                    r0 = q * 512
                    nc.sync.dma_start(
                        wqk_a[:, 4 * q : 4 * q + 4, :],
                        wqk[r0 : r0 + 512].rearrange("(ho p) c -> p ho c", p=D),
                    )
                    nc.sync.dma_start(
                        x_a0[:, 4 * q : 4 * q + 4, :],
                        x_t[r0 : r0 + 512, 0:512].rearrange("(ho p) t -> p ho t", p=D),
                    )
                for q in range(4):
                    r0 = H // 2 + q * 512
                    nc.sync.dma_start(
                        wqk_b[:, 4 * q : 4 * q + 4, :],
                        wqk[r0 : r0 + 512].rearrange("(ho p) c -> p ho c", p=D),
                    )
                    nc.sync.dma_start(
                        x_b0[:, 4 * q : 4 * q + 4, :],
                        x_t[r0 : r0 + 512, 0:512].rearrange("(ho p) t -> p ho t", p=D),
                    )
                tb0 = tpool.tile([D, 4, 512], FP16, tag="tb", name="tb0")
                for ti in range(4):
                    nc.sync.dma_start(tb0[:, ti, :], tabs[ti, :, 0:512])
                nc.sync.dma_start(
                    wv_sb[:, 0 : NHT // 2, :],
                    wv[: H // 2].rearrange("(ho p) c -> p ho c", p=D),
                )
                nc.sync.dma_start(
                    wv_sb[:, NHT // 2 :, :],
                    wv[H // 2 :].rearrange("(ho p) c -> p ho c", p=D),
                )
                ones_mat = constp.tile([D, D], FP16, tag="ones_mat")
                nc.sync.dma_start(ones_mat[:], nc.inline_tensor(ones_mat_np, "ones_mat_c")[:])
                maskbin = constp.tile([D, 896], FP16, tag="maskbin")
                nc.sync.dma_start(maskbin[:], nc.inline_tensor(maskbin_np, "maskbin_c")[:])

                for tci in range(T // 512):
                    t0 = tci * 512
                    if tci == 0:
                        x_a, x_b, tb = x_a0, x_b0, tb0
                    else:
                        x_a = xpool.tile([D, NHT // 2, 512], FP16, tag="xA",
                                         name=f"x_a{tci}")
                        nc.sync.dma_start(
                            x_a[:],
                            x_t[: H // 2, t0 : t0 + 512].rearrange(
                                "(ho p) t -> p ho t", p=D
                            ),
                        )
                        x_b = xpool.tile([D, NHT // 2, 512], FP16, tag="xB",
                                         name=f"x_b{tci}")
                        nc.sync.dma_start(
                            x_b[:],
                            x_t[H // 2 :, t0 : t0 + 512].rearrange(
                                "(ho p) t -> p ho t", p=D
                            ),
                        )
                        tb = tpool.tile([D, 4, 512], FP16, tag="tb", name=f"tb{tci}")
                        for ti in range(4):
                            nc.sync.dma_start(tb[:, ti, :], tabs[ti, :, t0 : t0 + 512])

                    def x_at(h, x_a=x_a, x_b=x_b):
                        return x_a[:, h, :] if h < NHT // 2 else x_b[:, h - NHT // 2, :]

                    for ct in range(2 * CH // D):  # 0-3: q heads, 4-7: k heads
                        psum = ps1.tile([D, 512], F32, tag="ps1")
                        for h in range(NHT):
                            nc.tensor.matmul(
                                psum[:],
                                wqk_at(h)[:, ct * D : (ct + 1) * D],
                                x_at(h),
                                start=(h == 0),
                                stop=(h == NHT - 1),
                            )
                        is_q = ct < HEADS_PER_CORE
                        cos_i = 0 if is_q else 2
                        sin_i = 1 if is_q else 3
                        tmp1 = rpool.tile([D, 512], F32, tag="tmp1")
                        nc.vector.tensor_mul(tmp1[:], psum[:], tb[:, cos_i, :])
                        tmp2 = rpool.tile([D, 512], F32, tag="tmp2")
                        nc.vector.tensor_mul(
                            tmp2[0:64, :], psum[64:128, :], tb[0:64, sin_i, :]
                        )
                        nc.vector.tensor_mul(
                            tmp2[64:128, :], psum[0:64, :], tb[64:128, sin_i, :]
                        )
                        qk_16 = qkout.tile([D, 512], FP16, tag="qk_16")
                        nc.vector.tensor_add(qk_16[:], tmp1[:], tmp2[:])
                        head = ct % HEADS_PER_CORE
                        dst = qt_d if is_q else kt_d
                        nc.sync.dma_start(
                            dst[head * D : (head + 1) * D, t0 : t0 + 512], qk_16[:]
                        )

                    for ts in range(4):  # V: [t, ch] layout
                        psum = ps1.tile([D, 512], F32, tag="ps1", name="psum_v")
                        for h in range(NHT):
                            nc.tensor.matmul(
                                psum[:],
                                x_at(h)[:, ts * D : (ts + 1) * D],
                                wv_sb[:, h, :],
                                start=(h == 0),
                                stop=(h == NHT - 1),
                            )
                        v_16 = qkout.tile([D, CH], FP16, tag="v_16")
                        nc.scalar.activation(
                            v_16[:], psum[:], mybir.ActivationFunctionType.Copy
                        )
                        nc.sync.dma_start(
                            v_d[t0 + ts * D : t0 + (ts + 1) * D, :], v_16[:]
                        )

            wpool_bv.release()
            kqv = tc.alloc_tile_pool(name="kqv", bufs=6)

            def load_head(b, head, pool):
                k_sb = pool.tile([D, S], FP16, tag="k_sb", name=f"k_sb{b}_{head}")
                nc.sync.dma_start(
                    k_sb[:], kt_d[head * D : (head + 1) * D, b * S : (b + 1) * S]
                )
                q_sb = pool.tile([D, S], FP16, tag="q_sb", name=f"q_sb{b}_{head}")
                nc.sync.dma_start(
                    q_sb[:], qt_d[head * D : (head + 1) * D, b * S : (b + 1) * S]
                )
                v_sb = pool.tile([D, KT, D], FP16, tag="v_sb", name=f"v_sb{b}_{head}")
                nc.sync.dma_start(
                    v_sb[:],
                    v_d[b * S : (b + 1) * S, head * D : (head + 1) * D].rearrange(
                        "(o p) c -> p o c", p=D
                    ),
                )
                return k_sb, q_sb, v_sb

            # b0 heads 0/1 live in the pre-pool (virgin SBUF, loads flow
            # during the stage-1 tensor tail); everything else in kqv.
            # All loads are emitted up front so none of them queue behind
            # the AllGather rings in the DMA queues.
            tiles = {}
            tiles[(0, 0)] = load_head(0, 0, kqv_pre)
            tiles[(0, 1)] = load_head(0, 1, kqv_pre)
            for bb, hh in [(0, 2), (0, 3), (1, 0), (1, 1), (1, 2), (1, 3)]:
                tiles[(bb, hh)] = load_head(bb, hh, kqv)

            # ------------- stage 2: attention + chunked AllGather -------------
            with (
                tc.tile_pool(name="ppool", bufs=10) as ppool,
                tc.tile_pool(name="accp", bufs=4) as accp,
                tc.tile_pool(name="mpool", bufs=2) as mpool,
                tc.tile_pool(name="epi", bufs=2) as epi,
                tc.tile_pool(name="ps_s", bufs=2, space="PSUM") as ps_s,
                tc.tile_pool(name="ps_av", bufs=3, space="PSUM") as ps_av,
                tc.tile_pool(name="ps_sbc", bufs=1, space="PSUM") as ps_sbc,
            ):
                qe, qm = [], []

                def drain(q, keep=0):
                    while len(q) > keep:
                        q.pop(0)()

                def group_gen(b, head, qc, tiles):
                    k_sb, q_sb, v_sb = tiles
                    diag = [kt for kt in range(KT) if ops[b, qc, kt] == DIAG]
                    dmam = [kt for kt in range(KT) if ops[b, qc, kt] == DMAMASK]
                    plain = [kt for kt in range(KT) if ops[b, qc, kt] == NOMASK]
                    kts = diag + dmam + plain
                    n_kt = len(kts)
                    psum_av = ps_av.tile([D, 512], F32, tag="av", name=f"av{b}{head}{qc}")
                    acc2 = accp.tile([D, 2, 512], FP16, tag="acc2",
                                     name=f"acc2{b}{head}{qc}")
                    gi = 0
                    for p0 in range(0, n_kt, 2):
                        pair = kts[p0 : p0 + 2]
                        np_ = len(pair)
                        psum_s = ps_s.tile(
                            [D, 2, 512], F32, tag="s", name=f"s{b}{head}{qc}{p0}"
                        )
                        diag_slices = []
                        for sl, kt in enumerate(pair):
                            nc.tensor.matmul(
                                psum_s[:, sl, :],
                                k_sb[:, kt * D : (kt + 1) * D],
                                q_sb[:, qc * 512 : (qc + 1) * 512],
                                start=True,
                                stop=True,
                            )
                            op = ops[b, qc, kt]
                            if op == DIAG:
                                diag_slices.append((sl, kt * D - qc * 512))
                            elif op == DMAMASK:
                                mt = mpool.tile([D, 512], F32, tag="mt", name="mt")
                                nc.sync.dma_start(
                                    mt[:],
                                    maskT[
                                        b,
                                        kt * D : (kt + 1) * D,
                                        qc * 512 : (qc + 1) * 512,
                                    ],
                                )
                                nc.vector.tensor_add(
                                    psum_s[:, sl, :], psum_s[:, sl, :], mt[:]
                                )

                        cell = []
                        first = p0 == 0

                        def exp_step(psum_s=psum_s, np_=np_, cell=cell,
                                     diag_slices=diag_slices, first=first,
                                     acc2=acc2):
                            pexp = ppool.tile([D, 2, 512], FP16, tag="pexp", name="pexp")
                            nc.scalar.activation(
                                pexp[:, 0:np_, :],
                                psum_s[:, 0:np_, :],
                                mybir.ActivationFunctionType.Exp,
                            )
                            for sl, delta in diag_slices:
                                nc.gpsimd.tensor_mul(
                                    pexp[:, sl, :],
                                    pexp[:, sl, :],
                                    maskbin[:, 384 - delta : 896 - delta],
                                )
                            eng = nc.vector
                            if first:
                                eng.tensor_copy(
                                    acc2[:, 0:np_, :], pexp[:, 0:np_, :]
                                )
                            else:
                                eng.tensor_add(
                                    acc2[:, 0:np_, :], acc2[:, 0:np_, :],
                                    pexp[:, 0:np_, :],
                                )
                            cell.append(pexp)

                        def mm_step(
                            psum_av=psum_av,
                            gi=gi,
                            pair=pair,
                            n_kt=n_kt,
                            v_sb=v_sb,
                            cell=cell,
                        ):
                            pexp = cell[0]
                            for sl, kt in enumerate(pair):
                                i = gi + sl
                                nc.tensor.matmul(
                                    psum_av[:],
                                    v_sb[:, kt, :],
                                    pexp[:, sl, :],
                                    start=(i == 0),
                                    stop=(i == n_kt - 1),
                                )

                        gi += np_
                        yield (exp_step, mm_step)

                    def epilogue(psum_av=psum_av, acc2=acc2, n_kt=n_kt):
                        n_sl = 2 if n_kt >= 2 else 1
                        psum_sbc = ps_sbc.tile(
                            [D, 512], F32, tag="sbc", name=f"sbc{b}{head}{qc}"
                        )
                        for sl in range(n_sl):
                            nc.tensor.matmul(
                                psum_sbc[:],
                                ones_mat[:],
                                acc2[:, sl, :],
                                start=(sl == 0),
                                stop=(sl == n_sl - 1),
                            )
                        bc_sb = epi.tile([D, 512], F32, tag="bc_sb", name="bc_sb")
                        nc.vector.reciprocal_approx_fast(bc_sb[:], psum_sbc[:])
                        attn_sb = epi.tile([D, 512], FP16, tag="attn_sb", name="attn_sb")
                        nc.vector.tensor_mul(attn_sb[:], psum_av[:], bc_sb[:])
                        for j in range(2):
                            nc.sync.dma_start(
                                a2a_in[b][
                                    2 * qc + j,
                                    head * D : (head + 1) * D,
                                    :,
                                ],
                                attn_sb[:, j * TSL : (j + 1) * TSL],
                            )

                    yield (None, epilogue)

                def run_section(gens):
                    active = []
                    queue = list(gens)
                    while queue or active:
                        while len(active) < 2 and queue:
                            active.append(queue.pop(0))
                        progressed = []
                        for g in active:
                            item = next(g, None)
                            if item is None:
                                continue
                            e, m = item
                            if e is not None:
                                qe.append(e)
                            qm.append(m)
                            drain(qe, 2)
                            drain(qm, 10)
                            progressed.append(g)
                        active = [g for g in active if g in progressed]
                    drain(qe)
                    drain(qm)

                for b in range(B):
                    run_section(
                        [
                            group_gen(b, head, qc, tiles[(b, head)])
                            for qc in (3, 2, 1, 0)
                            for head in range(HEADS_PER_CORE)
                        ]
                    )
                    nc.gpsimd.collective_compute(
                        "AllToAll",
                        mybir.AluOpType.bypass,
                        replica_groups=rg,
                        ins=[a2a_in[b].opt()],
                        outs=[a2a_out[b].opt()],
                    )

            kqv.release()
            kqv_pre.release()

            # ------------- stage 3: token-sharded o_proj, streamed W_o --------
            # Each core computes its own 256-token slice (per batch) of the
            # FULL-width o_proj from the AllToAll'd activations; W_o streams
            # through SBUF in 512-column chunks with no collective deps.
            with (
                tc.tile_pool(name="apool", bufs=1) as apool,
                tc.tile_pool(name="wop", bufs=3) as wop,
                tc.tile_pool(name="oppool", bufs=3) as oppool,
                tc.tile_pool(name="ps3", bufs=3, space="PSUM") as ps3,
            ):
                def load_att_f(b):
                    af = apool.tile([D, NHT, TSL], FP16, tag=f"att_f{b}",
                                    name=f"att_f{b}")
                    nc.sync.dma_start(
                        af[:],
                        a2a_out[b][:, :, :].rearrange(
                            "k (co p) t -> p (k co) t", p=D
                        ),
                    )
                    return af

                att_f = [load_att_f(0), None]
                NOC = H // 512                   # 8 o-column chunks
                for b in range(B):
                    for oc in range(NOC):
                        wo_c = wop.tile([D, NHT, 512], FP16, tag="wo_c")
                        nc.sync.dma_start(
                            wo_c[:],
                            wo[:, oc * 512 : (oc + 1) * 512].rearrange(
                                "(co p) o -> p co o", p=D
                            ),
                        )
                        if b == 0 and oc == 0:
                            # emitted after the first W_o chunk so its wait on
                            # the b1 AllToAll can't block the weight stream
                            att_f[1] = load_att_f(1)
                        for tl in range(TSL // D):
                            psum_o = ps3.tile([D, 512], F32, tag="ps_o")
                            for ct in range(NHT):
                                nc.tensor.matmul(
                                    psum_o[:],
                                    att_f[b][:, ct, tl * D : (tl + 1) * D],
                                    wo_c[:, ct, :],
                                    start=(ct == 0),
                                    stop=(ct == NHT - 1),
                                )
                            o_sb = oppool.tile([D, 512], F32, tag="o_sb")
                            nc.scalar.activation(
                                o_sb[:],
                                psum_o[:],
                                mybir.ActivationFunctionType.Copy,
                            )
                            nc.sync.dma_start(
                                out[
                                    b * TSL + tl * D : b * TSL + (tl + 1) * D,
                                    oc * 512 : (oc + 1) * 512,
                                ],
                                o_sb[:],
                            )

    nc.compile()
    return nc, maskT is not None


def kernel(hidden_states, attention_mask, position_ids, W_pack, W_o):
    _ensure_trace_hook()
    hidden_states = np.asarray(hidden_states, dtype=np.float32)
    attention_mask = np.asarray(attention_mask, dtype=np.float32)
    position_ids = np.asarray(position_ids)
    W_pack = np.asarray(W_pack, dtype=np.float32)
    W_o = np.asarray(W_o, dtype=np.float32)

    ops, need_dma = _classify_mask(attention_mask)

    key = (ops.tobytes(), need_dma)
    if key not in _cache:
        _cache.clear()
        _cache[key] = _build(ops, need_dma)
    nc, has_mask_param = _cache[key]

    # ---- host-side prep ----
    X_T = np.ascontiguousarray(hidden_states.reshape(T, H).T).astype(np.float16)

    # RoPE tables (position-gathered), transposed to [d, t]; scale folded into Q's.
    pos = position_ids.reshape(T).astype(np.float32)
    inv_freq = (1.0 / (BASE ** (np.arange(0, D, 2, dtype=np.float32) / D))).astype(
        np.float32
    )
    ang = pos[:, None] * inv_freq[None, :]          # [T, 64]
    ang = np.concatenate([ang, ang], axis=1)         # [T, 128]
    cos = np.cos(ang).astype(np.float32)
    sin = np.sin(ang).astype(np.float32)
    sin_signed = sin.copy()
    sin_signed[:, :64] *= -1.0                       # rows d<64 multiply -q[d+64]
    isd = np.float32(1.0 / math.sqrt(D))
    tabs = np.stack(
        [
            (cos * isd).T,
            (sin_signed * isd).T,
            cos.T,
            sin_signed.T,
        ]
    ).astype(np.float16)                             # [4, 128, T]
    tabs = np.ascontiguousarray(tabs)

    maskT_np = None
    if has_mask_param:
        maskT_np = np.ascontiguousarray(
            np.transpose(attention_mask[:, 0], (0, 2, 1))
        ).astype(np.float32)                         # [B, S(k), S(q)]

    wo_full = np.ascontiguousarray(W_o.T).astype(np.float16)  # [H(in ch), H(out)]
    in_maps = []
    for c in range(N_CORES):
        qr = slice(c * CH, (c + 1) * CH)
        kr = slice(H + c * CH, H + (c + 1) * CH)
        vr = slice(2 * H + c * CH, 2 * H + (c + 1) * CH)
        wqk_c = np.ascontiguousarray(
            np.concatenate([W_pack[qr], W_pack[kr]], axis=0).T
        ).astype(np.float16)                         # [H, 1024]
        wv_c = np.ascontiguousarray(W_pack[vr].T).astype(np.float16)  # [H, 512]
        m = {"x_t": X_T, "wqk": wqk_c, "wv": wv_c, "wo": wo_full, "tabs": tabs}
        if has_mask_param:
            m["maskT"] = maskT_np
        in_maps.append(m)

    import os

    trace = bool(os.environ.get("BASS_TRACE"))
    res = run_bass_kernel_spmd(
        nc, in_maps, core_ids=list(range(N_CORES)), trace=trace
    )
    last_run_info["exec_time_ns"] = res.exec_time_ns
    last_run_info["profile_json"] = getattr(res, "profile_json", None)

    TSL = S // N_CORES
    full = np.empty((B, S, H), dtype=np.float32)
    for c in range(N_CORES):
        o = res.results[c]["out"].reshape(B, TSL, H)
        full[:, c * TSL : (c + 1) * TSL, :] = o
    return full


# revision 56
# speedup vs baseline: 1.0434x; 1.0252x over previous
"""Distributed Bass kernel for nn_Attention (dense transformer attention block).

Tensor-parallel over heads across 8 TRN2 NeuronCores:
  - each core owns 4 heads: its slice of W_pack (QKV) and the matching
    512 input channels of o_proj,
  - attention (RoPE + causal softmax) is computed fully locally per head,
  - attention outputs (UNNORMALIZED, with per-row softmax denominators kept
    separately) are AllGathered in fp16 in two q-halves per batch; each core
    computes a distinct 512-column slice of the o_proj output with the
    1/denominator folded into the psum evacuation as a per-partition scale.

Compute dtype: fp16 matmuls with f32 PSUM accumulation, f32 softmax logits.
Scores are computed transposed ([k, q] layout) so the softmax exp writes
P^T directly and PV needs no transpose. Softmax denominators come from
DVE pair-accumulation of the exp tiles into an fp16 accumulator plus two
small ones-matmuls per group (instead of one ones-matmul per k-tile),
keeping the tensor engine (the global bottleneck, power-throttled to
~81% duty) on pure model FLOPs.

Schedule: stage1 (QKV+RoPE, PE-saturated) -> attention b0 (qc 3,2 ->
AllGather half1, qc 1,0 -> AllGather half0) -> attention b1 (kqv tiles
prefetched) -> o_proj b0 -> o_proj b1, with the AllGathers flying under
attention/o_proj compute.
"""

import sys
import types
import math

sys.path.insert(0, "/opt/trn_rl_repo")

import numpy as np

from concourse import bacc, tile, mybir
from concourse.bass_utils import run_bass_kernel_spmd

FP16 = mybir.dt.float16
F32 = mybir.dt.float32

B = 2
S = 2048
H = 4096
NH = 32
D = 128
T = B * S
N_CORES = 8
HEADS_PER_CORE = NH // N_CORES          # 4
CH = HEADS_PER_CORE * D                 # 512 channels per core
BASE = 10000.0
NEG = np.float32(np.finfo(np.float32).min)

# mask-block ops (per [q-chunk=512, k-tile=128] block, scoresT layout)
SKIP, NOMASK, DIAG, DMAMASK = 0, 1, 2, 3

QC = S // 512                            # 4 q-chunks per batch
KT = S // 128                            # 16 k-tiles per batch

_cache = {}
last_run_info = {}


def _ensure_trace_hook():
    """Register the NTFF profile hook missing from this image's antenv."""
    if "antenv.axon_hooks" in sys.modules:
        return
    try:
        from trn_agent_boot.trn_boot import _ntff_profile_via_ctypes

        hook = _ntff_profile_via_ctypes("/opt/axon/libaxon_pjrt.so")
        mod = types.ModuleType("antenv.axon_hooks")
        mod.get_axon_ntff_profile_hook = lambda: hook
        mod.set_axon_ntff_profile_hook = lambda h: None
        sys.modules["antenv.axon_hooks"] = mod
        from concourse import bass_utils

        bass_utils.upload_artifacts = lambda tmpdir: tmpdir
    except Exception:
        pass


def _classify_mask(mask):
    """Per (b, q-chunk 512, k-tile 128) block op for the scoresT layout."""
    ops = np.empty((B, QC, KT), dtype=np.int32)
    karr = np.arange(128)
    qarr = np.arange(512)
    need_dma = False
    for b in range(B):
        mb = np.asarray(mask[b, 0])
        for qc in range(QC):
            qs = qc * 512
            for kt in range(KT):
                ks = kt * 128
                sub = mb[qs : qs + 512, ks : ks + 128]  # [q, k]
                if np.all(sub <= -1e30):
                    ops[b, qc, kt] = SKIP
                elif not sub.any():
                    ops[b, qc, kt] = NOMASK
                else:
                    delta = ks - qs
                    if 0 <= delta <= 384:
                        pat = np.where(
                            (delta + karr[None, :]) > qarr[:, None], NEG, np.float32(0)
                        )
                        if np.array_equal(sub, pat):
                            ops[b, qc, kt] = DIAG
                            continue
                    ops[b, qc, kt] = DMAMASK
                    need_dma = True
    return ops, need_dma


def _build(ops, need_dma):
    nc = bacc.Bacc(None, target_bir_lowering=False)

    x_t = nc.declare_dram_parameter("x_t", [H, T], FP16, isOutput=False)
    wqk = nc.declare_dram_parameter("wqk", [H, 2 * CH], FP16, isOutput=False)
    wv = nc.declare_dram_parameter("wv", [H, CH], FP16, isOutput=False)
    wo = nc.declare_dram_parameter("wo", [H, CH], FP16, isOutput=False)
    tabs = nc.declare_dram_parameter("tabs", [4, D, T], FP16, isOutput=False)
    maskT = None
    if need_dma:
        maskT = nc.declare_dram_parameter("maskT", [B, S, S], F32, isOutput=False)
    out = nc.declare_dram_parameter("out", [T, CH], FP16, isOutput=True)

    ones_mat_np = np.ones((D, D), dtype=np.float16)
    # maskbin[k, j] = 0 if k > j - 384 else 1  (slice [384-delta : 896-delta])
    j = np.arange(896)
    maskbin_np = np.where(np.arange(D)[:, None] > (j[None, :] - 384), 0.0, 1.0).astype(
        np.float16
    )

    rg = [list(range(N_CORES))]
    NHT = H // D  # 32 h-tiles

    with tile.TileContext(nc) as tc:
        with (
            tc.tile_pool(name="dram", bufs=1, space="DRAM") as dram,
            tc.tile_pool(name="const", bufs=1) as constp,
            tc.tile_pool(name="wpool", bufs=1) as wpool,
        ):
            wpool_bv = tc.alloc_tile_pool(name="wpool_bv", bufs=1)
            kqv_pre = tc.alloc_tile_pool(name="kqv_pre", bufs=2, side="right")
            qt_d = dram.tile([CH, T], FP16, tag="qt_d")
            kt_d = dram.tile([CH, T], FP16, tag="kt_d")
            v_d = dram.tile([T, CH], FP16, tag="v_d")
            # per-batch, per-q-half attention staging for chunked AllGather
            att_loc = [
                [
                    dram.tile([CH, 1024], FP16, tag=f"att_loc{b}_{h}",
                              name=f"att_loc{b}_{h}")
                    for h in range(2)
                ]
                for b in range(B)
            ]
            att_all = [
                [
                    dram.tile([N_CORES * CH, 1024], FP16, addr_space="Shared",
                              tag=f"att_all{b}_{h}", name=f"att_all{b}_{h}")
                    for h in range(2)
                ]
                for b in range(B)
            ]

            # ------------- stage 1: QKV projection + RoPE (single X pass) ----
            wqk_a = wpool.tile([D, NHT // 2, 2 * CH], FP16, tag="wqkA", name="wqk_a")
            wqk_b = wpool_bv.tile([D, NHT // 2, 2 * CH], FP16, tag="wqkB", name="wqk_b")
            wv_sb = wpool_bv.tile([D, NHT, CH], FP16, tag="wv")

            def wqk_at(h):
                return wqk_a[:, h, :] if h < NHT // 2 else wqk_b[:, h - NHT // 2, :]

            with (
                tc.tile_pool(name="xpool", bufs=2) as xpool,
                tc.tile_pool(name="tpool", bufs=1) as tpool,
                tc.tile_pool(name="rpool", bufs=2) as rpool,
                tc.tile_pool(name="qkout", bufs=2) as qkout,
                tc.tile_pool(name="ps1", bufs=6, space="PSUM") as ps1,
            ):
                # chunk-0 x tiles allocated up front so their DMAs can be
                # interleaved with the weight quarters (fast PE start)
                x_a0 = xpool.tile([D, NHT // 2, 512], FP16, tag="xA", name="x_a0")
                x_b0 = xpool.tile([D, NHT // 2, 512], FP16, tag="xB", name="x_b0")
                for q in range(4):
                    r0 = q * 512
                    nc.sync.dma_start(
                        wqk_a[:, 4 * q : 4 * q + 4, :],
                        wqk[r0 : r0 + 512].rearrange("(ho p) c -> p ho c", p=D),
                    )
                    nc.sync.dma_start(
                        x_a0[:, 4 * q : 4 * q + 4, :],
                        x_t[r0 : r0 + 512, 0:512].rearrange("(ho p) t -> p ho t", p=D),
                    )
                for q in range(4):
                    r0 = H // 2 + q * 512
                    nc.sync.dma_start(
                        wqk_b[:, 4 * q : 4 * q + 4, :],
                        wqk[r0 : r0 + 512].rearrange("(ho p) c -> p ho c", p=D),
                    )
                    nc.sync.dma_start(
                        x_b0[:, 4 * q : 4 * q + 4, :],
                        x_t[r0 : r0 + 512, 0:512].rearrange("(ho p) t -> p ho t", p=D),
                    )
                tb0 = tpool.tile([D, 4, 512], FP16, tag="tb", name="tb0")
                for ti in range(4):
                    nc.sync.dma_start(tb0[:, ti, :], tabs[ti, :, 0:512])
                nc.sync.dma_start(
                    wv_sb[:, 0 : NHT // 2, :],
                    wv[: H // 2].rearrange("(ho p) c -> p ho c", p=D),
                )
                nc.sync.dma_start(
                    wv_sb[:, NHT // 2 :, :],
                    wv[H // 2 :].rearrange("(ho p) c -> p ho c", p=D),
                )
                ones_mat = constp.tile([D, D], FP16, tag="ones_mat")
                nc.sync.dma_start(ones_mat[:], nc.inline_tensor(ones_mat_np, "ones_mat_c")[:])
                maskbin = constp.tile([D, 896], FP16, tag="maskbin")
                nc.sync.dma_start(maskbin[:], nc.inline_tensor(maskbin_np, "maskbin_c")[:])

                for tci in range(T // 512):
                    t0 = tci * 512
                    if tci == 0:
                        x_a, x_b, tb = x_a0, x_b0, tb0
                    else:
                        x_a = xpool.tile([D, NHT // 2, 512], FP16, tag="xA",
                                         name=f"x_a{tci}")
                        nc.sync.dma_start(
                            x_a[:],
                            x_t[: H // 2, t0 : t0 + 512].rearrange(
                                "(ho p) t -> p ho t", p=D
                            ),
                        )
                        x_b = xpool.tile([D, NHT // 2, 512], FP16, tag="xB",
                                         name=f"x_b{tci}")
                        nc.sync.dma_start(
                            x_b[:],
                            x_t[H // 2 :, t0 : t0 + 512].rearrange(
                                "(ho p) t -> p ho t", p=D
                            ),
                        )
                        tb = tpool.tile([D, 4, 512], FP16, tag="tb", name=f"tb{tci}")
                        for ti in range(4):
                            nc.sync.dma_start(tb[:, ti, :], tabs[ti, :, t0 : t0 + 512])

                    def x_at(h, x_a=x_a, x_b=x_b):
                        return x_a[:, h, :] if h < NHT // 2 else x_b[:, h - NHT // 2, :]

                    for ct in range(2 * CH // D):  # 0-3: q heads, 4-7: k heads
                        psum = ps1.tile([D, 512], F32, tag="ps1")
                        for h in range(NHT):
                            nc.tensor.matmul(
                                psum[:],
                                wqk_at(h)[:, ct * D : (ct + 1) * D],
                                x_at(h),
                                start=(h == 0),
                                stop=(h == NHT - 1),
                            )
                        is_q = ct < HEADS_PER_CORE
                        cos_i = 0 if is_q else 2
                        sin_i = 1 if is_q else 3
                        tmp1 = rpool.tile([D, 512], F32, tag="tmp1")
                        nc.vector.tensor_mul(tmp1[:], psum[:], tb[:, cos_i, :])
                        tmp2 = rpool.tile([D, 512], F32, tag="tmp2")
                        nc.vector.tensor_mul(
                            tmp2[0:64, :], psum[64:128, :], tb[0:64, sin_i, :]
                        )
                        nc.vector.tensor_mul(
                            tmp2[64:128, :], psum[0:64, :], tb[64:128, sin_i, :]
                        )
                        qk_16 = qkout.tile([D, 512], FP16, tag="qk_16")
                        nc.vector.tensor_add(qk_16[:], tmp1[:], tmp2[:])
                        head = ct % HEADS_PER_CORE
                        dst = qt_d if is_q else kt_d
                        nc.sync.dma_start(
                            dst[head * D : (head + 1) * D, t0 : t0 + 512], qk_16[:]
                        )

                    for ts in range(4):  # V: [t, ch] layout
                        psum = ps1.tile([D, 512], F32, tag="ps1", name="psum_v")
                        for h in range(NHT):
                            nc.tensor.matmul(
                                psum[:],
                                x_at(h)[:, ts * D : (ts + 1) * D],
                                wv_sb[:, h, :],
                                start=(h == 0),
                                stop=(h == NHT - 1),
                            )
                        v_16 = qkout.tile([D, CH], FP16, tag="v_16")
                        nc.scalar.activation(
                            v_16[:], psum[:], mybir.ActivationFunctionType.Copy
                        )
                        nc.sync.dma_start(
                            v_d[t0 + ts * D : t0 + (ts + 1) * D, :], v_16[:]
                        )

            wpool_bv.release()
            kqv = tc.alloc_tile_pool(name="kqv", bufs=6)

            def load_head(b, head, pool):
                k_sb = pool.tile([D, S], FP16, tag="k_sb", name=f"k_sb{b}_{head}")
                nc.sync.dma_start(
                    k_sb[:], kt_d[head * D : (head + 1) * D, b * S : (b + 1) * S]
                )
                q_sb = pool.tile([D, S], FP16, tag="q_sb", name=f"q_sb{b}_{head}")
                nc.sync.dma_start(
                    q_sb[:], qt_d[head * D : (head + 1) * D, b * S : (b + 1) * S]
                )
                v_sb = pool.tile([D, KT, D], FP16, tag="v_sb", name=f"v_sb{b}_{head}")
                nc.sync.dma_start(
                    v_sb[:],
                    v_d[b * S : (b + 1) * S, head * D : (head + 1) * D].rearrange(
                        "(o p) c -> p o c", p=D
                    ),
                )
                return k_sb, q_sb, v_sb

            # b0 heads 0/1 live in the pre-pool (virgin SBUF, loads flow
            # during the stage-1 tensor tail); everything else in kqv.
            # All loads are emitted up front so none of them queue behind
            # the AllGather rings in the DMA queues.
            tiles = {}
            tiles[(0, 0)] = load_head(0, 0, kqv_pre)
            tiles[(0, 1)] = load_head(0, 1, kqv_pre)
            for bb, hh in [(0, 2), (0, 3), (1, 0), (1, 1), (1, 2), (1, 3)]:
                tiles[(bb, hh)] = load_head(bb, hh, kqv)

            # ------------- stage 2: attention + chunked AllGather -------------
            with (
                tc.tile_pool(name="ppool", bufs=10) as ppool,
                tc.tile_pool(name="accp", bufs=4) as accp,
                tc.tile_pool(name="mpool", bufs=2) as mpool,
                tc.tile_pool(name="epi", bufs=2) as epi,
                tc.tile_pool(name="ps_s", bufs=2, space="PSUM") as ps_s,
                tc.tile_pool(name="ps_av", bufs=3, space="PSUM") as ps_av,
                tc.tile_pool(name="ps_sbc", bufs=1, space="PSUM") as ps_sbc,
            ):
                qe, qm = [], []

                def drain(q, keep=0):
                    while len(q) > keep:
                        q.pop(0)()

                def group_gen(b, head, qc, tiles):
                    k_sb, q_sb, v_sb = tiles
                    diag = [kt for kt in range(KT) if ops[b, qc, kt] == DIAG]
                    dmam = [kt for kt in range(KT) if ops[b, qc, kt] == DMAMASK]
                    plain = [kt for kt in range(KT) if ops[b, qc, kt] == NOMASK]
                    kts = diag + dmam + plain
                    n_kt = len(kts)
                    psum_av = ps_av.tile([D, 512], F32, tag="av", name=f"av{b}{head}{qc}")
                    acc2 = accp.tile([D, 2, 512], FP16, tag="acc2",
                                     name=f"acc2{b}{head}{qc}")
                    gi = 0
                    for p0 in range(0, n_kt, 2):
                        pair = kts[p0 : p0 + 2]
                        np_ = len(pair)
                        psum_s = ps_s.tile(
                            [D, 2, 512], F32, tag="s", name=f"s{b}{head}{qc}{p0}"
                        )
                        diag_slices = []
                        for sl, kt in enumerate(pair):
                            nc.tensor.matmul(
                                psum_s[:, sl, :],
                                k_sb[:, kt * D : (kt + 1) * D],
                                q_sb[:, qc * 512 : (qc + 1) * 512],
                                start=True,
                                stop=True,
                            )
                            op = ops[b, qc, kt]
                            if op == DIAG:
                                diag_slices.append((sl, kt * D - qc * 512))
                            elif op == DMAMASK:
                                mt = mpool.tile([D, 512], F32, tag="mt", name="mt")
                                nc.sync.dma_start(
                                    mt[:],
                                    maskT[
                                        b,
                                        kt * D : (kt + 1) * D,
                                        qc * 512 : (qc + 1) * 512,
                                    ],
                                )
                                nc.vector.tensor_add(
                                    psum_s[:, sl, :], psum_s[:, sl, :], mt[:]
                                )

                        cell = []
                        first = p0 == 0

                        def exp_step(psum_s=psum_s, np_=np_, cell=cell,
                                     diag_slices=diag_slices, first=first,
                                     acc2=acc2):
                            pexp = ppool.tile([D, 2, 512], FP16, tag="pexp", name="pexp")
                            nc.scalar.activation(
                                pexp[:, 0:np_, :],
                                psum_s[:, 0:np_, :],
                                mybir.ActivationFunctionType.Exp,
                            )
                            for sl, delta in diag_slices:
                                nc.gpsimd.tensor_mul(
                                    pexp[:, sl, :],
                                    pexp[:, sl, :],
                                    maskbin[:, 384 - delta : 896 - delta],
                                )
                            eng = nc.vector
                            if first:
                                eng.tensor_copy(
                                    acc2[:, 0:np_, :], pexp[:, 0:np_, :]
                                )
                            else:
                                eng.tensor_add(
                                    acc2[:, 0:np_, :], acc2[:, 0:np_, :],
                                    pexp[:, 0:np_, :],
                                )
                            cell.append(pexp)

                        def mm_step(
                            psum_av=psum_av,
                            gi=gi,
                            pair=pair,
                            n_kt=n_kt,
                            v_sb=v_sb,
                            cell=cell,
                        ):
                            pexp = cell[0]
                            for sl, kt in enumerate(pair):
                                i = gi + sl
                                nc.tensor.matmul(
                                    psum_av[:],
                                    v_sb[:, kt, :],
                                    pexp[:, sl, :],
                                    start=(i == 0),
                                    stop=(i == n_kt - 1),
                                )

                        gi += np_
                        yield (exp_step, mm_step)

                    def epilogue(psum_av=psum_av, acc2=acc2, n_kt=n_kt):
                        n_sl = 2 if n_kt >= 2 else 1
                        psum_sbc = ps_sbc.tile(
                            [D, 512], F32, tag="sbc", name=f"sbc{b}{head}{qc}"
                        )
                        for sl in range(n_sl):
                            nc.tensor.matmul(
                                psum_sbc[:],
                                ones_mat[:],
                                acc2[:, sl, :],
                                start=(sl == 0),
                                stop=(sl == n_sl - 1),
                            )
                        bc_sb = epi.tile([D, 512], F32, tag="bc_sb", name="bc_sb")
                        nc.vector.reciprocal_approx_fast(bc_sb[:], psum_sbc[:])
                        attn_sb = epi.tile([D, 512], FP16, tag="attn_sb", name="attn_sb")
                        nc.vector.tensor_mul(attn_sb[:], psum_av[:], bc_sb[:])
                        c0 = (qc % 2) * 512
                        nc.sync.dma_start(
                            att_loc[b][qc // 2][
                                head * D : (head + 1) * D, c0 : c0 + 512
                            ],
                            attn_sb[:],
                        )

                    yield (None, epilogue)

                def run_section(gens):
                    active = []
                    queue = list(gens)
                    while queue or active:
                        while len(active) < 2 and queue:
                            active.append(queue.pop(0))
                        progressed = []
                        for g in active:
                            item = next(g, None)
                            if item is None:
                                continue
                            e, m = item
                            if e is not None:
                                qe.append(e)
                            qm.append(m)
                            drain(qe, 2)
                            drain(qm, 10)
                            progressed.append(g)
                        active = [g for g in active if g in progressed]
                    drain(qe)
                    drain(qm)

                def gather_half(b, half):
                    nc.gpsimd.collective_compute(
                        "AllGather",
                        mybir.AluOpType.bypass,
                        replica_groups=rg,
                        ins=[att_loc[b][half].opt()],
                        outs=[att_all[b][half].opt()],
                    )

                for b in range(B):
                    for half in (1, 0):
                        qcs = (3, 2) if half == 1 else (1, 0)
                        run_section(
                            [
                                group_gen(b, head, qc, tiles[(b, head)])
                                for qc in qcs
                                for head in range(HEADS_PER_CORE)
                            ]
                        )
                        gather_half(b, half)

            kqv.release()
            kqv_pre.release()

            # ------------- stage 3: o_proj slice (fresh pools, deep rings) ----
            with (
                tc.tile_pool(name="apool", bufs=2) as apool,
                tc.tile_pool(name="wop", bufs=1) as wop,
                tc.tile_pool(name="oppool", bufs=3) as oppool,
                tc.tile_pool(name="ps3", bufs=3, space="PSUM") as ps3,
            ):
                wo_sb = wop.tile([D, NHT, CH], FP16, tag="wo_sb")
                nc.sync.dma_start(
                    wo_sb[:, 0 : NHT // 2, :],
                    wo[: H // 2].rearrange("(co p) o -> p co o", p=D),
                )
                nc.sync.dma_start(
                    wo_sb[:, NHT // 2 :, :],
                    wo[H // 2 :].rearrange("(co p) o -> p co o", p=D),
                )
                for b, half in ((0, 1), (0, 0), (1, 1), (1, 0)):
                    # whole q-half of the gathered activations: contiguous
                    # 2KB-row DMAs instead of 256B scatter-gather per tile
                    a_half = apool.tile([D, NHT, 1024], FP16, tag="a_half")
                    for cg in range(4):
                        nc.sync.dma_start(
                            a_half[:, cg * 8 : (cg + 1) * 8, :],
                            att_all[b][half][
                                cg * 8 * D : (cg + 1) * 8 * D, :
                            ].rearrange("(co p) t -> p co t", p=D),
                        )
                    for tl in range(8):
                        tt = half * 8 + tl
                        psum_o = ps3.tile([D, 512], F32, tag="ps_o")
                        for ct in range(NHT):
                            nc.tensor.matmul(
                                psum_o[:],
                                a_half[:, ct, tl * D : (tl + 1) * D],
                                wo_sb[:, ct, :],
                                start=(ct == 0),
                                stop=(ct == NHT - 1),
                            )
                        o_sb = oppool.tile([D, CH], FP16, tag="o_sb")
                        nc.scalar.activation(
                            o_sb[:],
                            psum_o[:],
                            mybir.ActivationFunctionType.Copy,
                        )
                        nc.sync.dma_start(
                            out[b * S + tt * D : b * S + (tt + 1) * D, :],
                            o_sb[:],
                        )

    nc.compile()
    return nc, maskT is not None


def kernel(hidden_states, attention_mask, position_ids, W_pack, W_o):
    _ensure_trace_hook()
    hidden_states = np.asarray(hidden_states, dtype=np.float32)
    attention_mask = np.asarray(attention_mask, dtype=np.float32)
    position_ids = np.asarray(position_ids)
    W_pack = np.asarray(W_pack, dtype=np.float32)
    W_o = np.asarray(W_o, dtype=np.float32)

    ops, need_dma = _classify_mask(attention_mask)

    key = (ops.tobytes(), need_dma)
    if key not in _cache:
        _cache.clear()
        _cache[key] = _build(ops, need_dma)
    nc, has_mask_param = _cache[key]

    # ---- host-side prep ----
    X_T = np.ascontiguousarray(hidden_states.reshape(T, H).T).astype(np.float16)

    # RoPE tables (position-gathered), transposed to [d, t]; scale folded into Q's.
    pos = position_ids.reshape(T).astype(np.float32)
    inv_freq = (1.0 / (BASE ** (np.arange(0, D, 2, dtype=np.float32) / D))).astype(
        np.float32
    )
    ang = pos[:, None] * inv_freq[None, :]          # [T, 64]
    ang = np.concatenate([ang, ang], axis=1)         # [T, 128]
    cos = np.cos(ang).astype(np.float32)
    sin = np.sin(ang).astype(np.float32)
    sin_signed = sin.copy()
    sin_signed[:, :64] *= -1.0                       # rows d<64 multiply -q[d+64]
    isd = np.float32(1.0 / math.sqrt(D))
    tabs = np.stack(
        [
            (cos * isd).T,
            (sin_signed * isd).T,
            cos.T,
            sin_signed.T,
        ]
    ).astype(np.float16)                             # [4, 128, T]
    tabs = np.ascontiguousarray(tabs)

    maskT_np = None
    if has_mask_param:
        maskT_np = np.ascontiguousarray(
            np.transpose(attention_mask[:, 0], (0, 2, 1))
        ).astype(np.float32)                         # [B, S(k), S(q)]

    in_maps = []
    for c in range(N_CORES):
        qr = slice(c * CH, (c + 1) * CH)
        kr = slice(H + c * CH, H + (c + 1) * CH)
        vr = slice(2 * H + c * CH, 2 * H + (c + 1) * CH)
        wqk_c = np.ascontiguousarray(
            np.concatenate([W_pack[qr], W_pack[kr]], axis=0).T
        ).astype(np.float16)                         # [H, 1024]
        wv_c = np.ascontiguousarray(W_pack[vr].T).astype(np.float16)  # [H, 512]
        wo_c = np.ascontiguousarray(W_o[c * CH : (c + 1) * CH, :].T).astype(
            np.float16
        )                                            # [H, 512]
        m = {"x_t": X_T, "wqk": wqk_c, "wv": wv_c, "wo": wo_c, "tabs": tabs}
        if has_mask_param:
            m["maskT"] = maskT_np
        in_maps.append(m)

    import os

    trace = bool(os.environ.get("BASS_TRACE"))
    res = run_bass_kernel_spmd(
        nc, in_maps, core_ids=list(range(N_CORES)), trace=trace
    )
    last_run_info["exec_time_ns"] = res.exec_time_ns
    last_run_info["profile_json"] = getattr(res, "profile_json", None)

    outs = [
        res.results[c]["out"].astype(np.float32).reshape(B, S, CH)
        for c in range(N_CORES)
    ]
    return np.concatenate(outs, axis=2)
